# revision 20
# baseline (speedup 1.0000x reference)
"""Trainium2 Bass kernel for nn_ALAttention (sparse local attention).

Sharding: 64 image rows split across 8 cores (8 query rows each). Each core
gets a host-gathered 16-row key/value slab; border cores (0 and 7) use
custom slab row arrangements with duplicated rows so that every query pair
t (128 queries = 2 image rows) attends only within slab chunks t..t+4
(host-asserted) -- this covers the clamped window/leg geometry at image
borders while keeping a uniform SPMD graph.

Per core: Q/K GEMMs run in fp8e4 with DoubleRow perf mode (w pre-scaled by
32 on host for fp8 range, undone in the copy-out; k bias dropped via
softmax shift-invariance; q scale folded into w/bias -- exact). V is
produced in bf16 directly in transposed [key, dim] layout by a GEMM with x
as the stationary operand (no PE transposes); v bias folds into the proj
bias on host. Attention per head pair is banded: each of 8 key chunks
scores only against the query pairs that attend it (widths 1,2,3,4,4,3,2,1
x 128). Scores are fp8 DoubleRow matmuls (second k-tile zeroed -- DR time
depends only on output columns), exp on Act, mask-mult on DVE/Pool, then
bf16 AV matmuls accumulate the numerator and (via an augmented ones column
in V) the softmax denominator into PSUM. Normalization: gather the two
denominator rows, fast reciprocal, partition-broadcast on Pool, two DVE
multiplies. Proj GEMM in bf16 with fused bias.

Scheduling: pair-granular software pipeline -- the masked-exp tiles of a
pair are buffered in SBUF so pair n's AV matmuls interleave into pair
n+1's score stream; V^T(b1), the b1 QKV block, and proj(b0) slot into the
streams as real PE filler; ot PSUM tiles are double-buffered so the
normalize chain overlaps the next pair.
"""
import os
import sys
import types
from collections import deque

sys.path.insert(0, "/opt/trn_rl_repo")

import numpy as np
import ml_dtypes

from concourse import bacc, tile, mybir
from concourse import bass_utils
from concourse.bass_utils import run_bass_kernel_spmd

F32 = mybir.dt.float32
BF16 = mybir.dt.bfloat16
FP8 = mybir.dt.float8e4
AF = mybir.ActivationFunctionType
ALU = mybir.AluOpType
DR = mybir.MatmulPerfMode.DoubleRow

B = 2
C = 384
HH = WW = 64
NCORES = 8
ROWS = 8
SLAB = 16
SCOLS = SLAB * WW      # 1024 slab key positions
QCOLS = ROWS * WW      # 512 queries per core
SCALE = float(64) ** -0.5
W8S = 32.0             # fp8 range pre-scale for w (undone in copy-out)
PAD = -1

W_J = [1, 2, 3, 4, 4, 3, 2, 1]            # query pairs attending key chunk j
T0_J = [max(0, j - 4) for j in range(8)]  # first attending pair
OFF1 = np.cumsum([0] + [w * 128 for w in W_J]).tolist()
MASKW = OFF1[-1]       # 2560 per half

LAST_EXEC_NS = None
LAST_TRACE = None
_NC_CACHE = {}


def _register_ntff_hook():
    if "antenv.axon_hooks" in sys.modules:
        return
    try:
        from trn_agent_boot.trn_boot import _ntff_profile_via_ctypes
        hook = _ntff_profile_via_ctypes("/opt/axon/libaxon_pjrt.so")
    except Exception:
        hook = None
    mod = types.ModuleType("antenv.axon_hooks")
    mod.get_axon_ntff_profile_hook = lambda: hook
    mod.set_axon_ntff_profile_hook = lambda h: None
    sys.modules["antenv.axon_hooks"] = mod
    bass_utils.upload_artifacts = lambda tmpdir: "local://skipped"


def slab_rows_for_core(i):
    if i == 0:
        return [6, 7, 8, 9] + list(range(0, 8)) + [8, 9, 10, 11]
    if i == 7:
        return [52, 53, 54, 55] + list(range(56, 64)) + [55, 56, 57, PAD]
    return list(range(8 * i - 4, 8 * i + 12))


def build_graph():
    nc = bacc.Bacc("TRN2", target_bir_lowering=False, debug=False,
                   num_devices=NCORES)

    xs_e = nc.dram_tensor("xs", [B, C, SCOLS], BF16, kind="ExternalInput").ap()
    x8_e = nc.dram_tensor("x8", [B, C, SCOLS], FP8, kind="ExternalInput").ap()
    # [K0 Q0 K1 Q1 K2 Q2] columns; x32, Q also xSCALE
    w8_e = nc.dram_tensor("w8", [C, 768], FP8, kind="ExternalInput").ap()
    wv_e = nc.dram_tensor("wv", [C, C], BF16, kind="ExternalInput").ap()
    bq_e = nc.dram_tensor("bq", [128, 3], F32, kind="ExternalInput").ap()
    wprojT_e = nc.dram_tensor("wprojT", [C, C], BF16, kind="ExternalInput").ap()
    bp_e = nc.dram_tensor("bp", [128, 3], F32, kind="ExternalInput").ap()
    mask_e = nc.dram_tensor("mask", [128, 2, MASKW], BF16,
                            kind="ExternalInput").ap()
    out_e = nc.dram_tensor("out", [B, C, QCOLS], F32, kind="ExternalOutput").ap()

    with tile.TileContext(nc) as tc:
        with (
            tc.tile_pool(name="const", bufs=1) as cpool,
            tc.tile_pool(name="esb", bufs=3) as epool,
            tc.tile_pool(name="sc", bufs=3) as scpool,
            tc.tile_pool(name="osb", bufs=2) as opool,
            tc.tile_pool(name="pmm", bufs=2, space="PSUM") as pmm,
            tc.tile_pool(name="pacc", bufs=2, space="PSUM") as pacc,
        ):
            # ---- constants / inputs ----
            x_sb = [cpool.tile([128, 3, SCOLS], BF16, tag=f"x{b}",
                               name=f"x_sb{b}") for b in range(B)]
            x8_sb = [cpool.tile([128, 3, SCOLS], FP8, tag=f"x8{b}",
                                name=f"x8_sb{b}") for b in range(B)]
            w8_sb = cpool.tile([128, 3, 768], FP8, tag="w8")
            wv_sb = cpool.tile([128, 3, C], BF16, tag="wv")
            wp_sb = cpool.tile([128, 3, C], BF16, tag="wproj")
            bq_sb = cpool.tile([128, 3], F32, tag="bq")
            bp_sb = cpool.tile([128, 3], F32, tag="bp")
            mask_sb = cpool.tile([128, 2, MASKW], BF16, tag="mask")

            qs = (nc.sync, nc.scalar, nc.gpsimd)
            for kk in range(3):
                q = qs[kk]
                r = slice(128 * kk, 128 * (kk + 1))
                q.dma_start(x8_sb[0][:, kk, :], x8_e[0, r, :])
                q.dma_start(w8_sb[:, kk, :], w8_e[r, :])
                q.dma_start(x_sb[0][:, kk, :], xs_e[0, r, :])
                q.dma_start(wv_sb[:, kk, :], wv_e[r, :])
            nc.sync.dma_start(mask_sb[:, :, 0:1280], mask_e[:, :, 0:1280])
            nc.scalar.dma_start(mask_sb[:, :, 1280:2560],
                                mask_e[:, :, 1280:2560])
            for kk in range(3):
                q = qs[kk]
                r = slice(128 * kk, 128 * (kk + 1))
                q.dma_start(x8_sb[1][:, kk, :], x8_e[1, r, :])
                q.dma_start(x_sb[1][:, kk, :], xs_e[1, r, :])
            nc.sync.dma_start(bq_sb[:], bq_e[:])
            nc.scalar.dma_start(bp_sb[:], bp_e[:])
            for kk in range(3):
                nc.gpsimd.dma_start(wp_sb[:, kk, :],
                                    wprojT_e[128 * kk:128 * (kk + 1), :])
            # pre-warm the scalar engine's EXP table
            warm_sb = cpool.tile([1, 1], F32, tag="warm")
            nc.gpsimd.memset(warm_sb[:], 0.0)
            nc.scalar.activation(warm_sb[:], warm_sb[:], AF.Exp)

            # qkv staging; k8/q8 carry a zeroed second DoubleRow k-tile
            k8_sb = [cpool.tile([128, 3, 2, SCOLS], FP8, tag=f"k{b}",
                                name=f"k8_sb{b}") for b in range(B)]
            q8_sb = [cpool.tile([128, 3, 2, QCOLS], FP8, tag=f"q{b}",
                                name=f"q8_sb{b}") for b in range(B)]
            for b in range(B):
                nc.gpsimd.memset(k8_sb[b][:, :, 1, :], 0.0)
                nc.gpsimd.memset(q8_sb[b][:, :, 1, :], 0.0)
            # v: [key-in-chunk, chunk j, half h, pair c, [64 dims|ones]]
            v_sb = [cpool.tile([128, 8, 2, 3, 65], BF16, tag=f"v{b}",
                               name=f"v_sb{b}") for b in range(B)]
            for b in range(B):
                for h in range(2):
                    nc.gpsimd.memset(v_sb[b][:, :, h, :, 64:65], 1.0)
            ocat = [[cpool.tile([128, QCOLS], BF16, tag=f"oc{b}{c}",
                                name=f"ocat{b}{c}") for c in range(3)]
                    for b in range(B)]

            # ---- unit emitters ----
            def unit_k(b, c, n, eng):
                def emit():
                    ps = pmm.tile([128, 512], F32, tag="st",
                                  name=f"kh{b}{c}{n}")
                    cols = slice(512 * n, 512 * (n + 1))
                    nc.tensor.matmul(ps[:], w8_sb[:, 0:2, 256 * c:256 * c + 128],
                                     x8_sb[b][:, 0:2, cols],
                                     start=True, stop=False, perf_mode=DR)
                    nc.tensor.matmul(ps[:], w8_sb[:, 2, 256 * c:256 * c + 128],
                                     x8_sb[b][:, 2, cols],
                                     start=False, stop=True)
                    if eng is nc.scalar:
                        eng.activation(k8_sb[b][:, c, 0, cols], ps[:],
                                       AF.Identity, scale=1.0 / W8S)
                    else:
                        eng.tensor_scalar(k8_sb[b][:, c, 0, cols], ps[:],
                                          1.0 / W8S, None, ALU.mult)
                return emit

            def unit_q(b, c):
                def emit():
                    ps = pmm.tile([128, 512], F32, tag="st",
                                  name=f"qh{b}{c}")
                    col0 = 256 * c + 128
                    nc.tensor.matmul(ps[:], w8_sb[:, 0:2, col0:col0 + 128],
                                     x8_sb[b][:, 0:2, 256:768],
                                     start=True, stop=False, perf_mode=DR)
                    nc.tensor.matmul(ps[:], w8_sb[:, 2, col0:col0 + 128],
                                     x8_sb[b][:, 2, 256:768],
                                     start=False, stop=True)
                    nc.scalar.activation(q8_sb[b][:, c, 0, :], ps[:],
                                         AF.Identity, bias=bq_sb[:, c:c + 1],
                                         scale=1.0 / W8S)
                return emit

            def unit_vt(b, pc, pool_tag="st"):
                def emit():
                    pool = pmm if pool_tag == "st" else pacc
                    ps = pool.tile([128, 3, 128], F32, tag=pool_tag,
                                   name=f"vt{b}{pc}", bufs=None
                                   if pool_tag == "st" else 4)
                    for kk in range(3):
                        nc.tensor.matmul(
                            ps[:], x_sb[b][:, kk, 128 * pc:128 * (pc + 1)],
                            wv_sb[:, kk, :],
                            start=(kk == 0), stop=(kk == 2))
                    for h in range(2):
                        nc.vector.tensor_copy(v_sb[b][:, pc, h, :, 0:64],
                                              ps[:, :, 64 * h:64 * h + 64])
                return emit

            def unit_proj(b, m, pool_tag="st"):
                def emit():
                    pool = pmm if pool_tag == "st" else pacc
                    ps = pool.tile([128, 512], F32, tag=pool_tag,
                                   name=f"pj{b}{m}", bufs=None
                                   if pool_tag == "st" else 4)
                    for kk in range(3):
                        nc.tensor.matmul(
                            ps[:], wp_sb[:, kk, 128 * m:128 * (m + 1)],
                            ocat[b][kk][:],
                            start=(kk == 0), stop=(kk == 2))
                    o = opool.tile([128, QCOLS], F32, tag="o", name=f"o{b}{m}")
                    nc.scalar.activation(o[:], ps[:], AF.Identity,
                                         bias=bp_sb[:, m:m + 1], scale=1.0)
                    (nc.sync, nc.scalar, nc.gpsimd)[m].dma_start(
                        out_e[b, 128 * m:128 * (m + 1), :], o[:])
                return emit

            # ---- attention: pair-granular software pipeline ----
            JORD = (3, 4, 2, 5, 1, 6, 0, 7)

            def scores_stream(b, c, es):
                for j in JORD:
                    w = W_J[j]
                    t0 = T0_J[j]
                    wk = w * 128
                    st = pmm.tile([128, 2, 512], F32, tag="st",
                                  name=f"st{b}{c}{j}")
                    for h in range(2):
                        hr = slice(64 * h, 64 * h + 64)
                        nc.tensor.matmul(
                            st[:, h, 0:wk],
                            k8_sb[b][hr, c, :, 128 * j:128 * (j + 1)],
                            q8_sb[b][hr, c, :, 128 * t0:128 * t0 + wk],
                            start=True, stop=True, perf_mode=DR)
                    e = epool.tile([128, 2, wk], BF16, tag=f"e{j}", bufs=2,
                                   name=f"e{b}{c}{j}")
                    nc.scalar.activation(e[:], st[:, :, 0:wk], AF.Exp)
                    meng = nc.gpsimd if j in (0, 7) else nc.vector
                    meng.tensor_tensor(e[:], e[:],
                                       mask_sb[:, :, OFF1[j]:OFF1[j] + wk],
                                       ALU.mult)
                    es.append(e)
                    yield

            def av_stream(b, c, es):
                ot = [pacc.tile([65, QCOLS], F32, tag="ot", bufs=4,
                                name=f"ot{b}{c}{h}") for h in range(2)]
                for idx, j in enumerate(JORD):
                    t0 = T0_J[j]
                    wk = W_J[j] * 128
                    for h in range(2):
                        nc.tensor.matmul(
                            ot[h][:, 128 * t0:128 * t0 + wk],
                            v_sb[b][:, j, h, c, :], es[idx][:, h, :],
                            start=(idx == 0), stop=(idx == 7),
                            skip_group_check=True)
                    yield
                # normalize: denominators sit at partition 64 of ot[h]
                srow = scpool.tile([1, 2 * QCOLS], F32, tag="srow",
                                   name=f"srow{b}{c}")
                for h in range(2):
                    nc.vector.tensor_copy(
                        srow[0:1, QCOLS * h:QCOLS * (h + 1)], ot[h][64:65, :])
                rr = scpool.tile([1, 2 * QCOLS], F32, tag="rr",
                                 name=f"rr{b}{c}")
                nc.vector.reciprocal_approx_fast(rr[:], srow[:])
                rb = [scpool.tile([64, QCOLS], F32, tag=f"rb{h}",
                                  name=f"rb{b}{c}{h}") for h in range(2)]
                for h in range(2):
                    nc.gpsimd.partition_broadcast(
                        rb[h][:], rr[0:1, QCOLS * h:QCOLS * (h + 1)])
                for h in range(2):
                    nc.vector.tensor_tensor(
                        ocat[b][c][64 * h:64 * h + 64, :], ot[h][0:64, :],
                        rb[h][:], ALU.mult)

            # ---- program ----
            a, v = nc.scalar, nc.vector
            for c in range(3):
                unit_k(0, c, 0, a)()
                unit_k(0, c, 1, v)()
                unit_q(0, c)()
            for pc in range(8):
                unit_vt(0, pc)()

            # P0 scores with b1 V^T interleaved (spare ot-tag psum slots)
            es = []
            sg = scores_stream(0, 0, es)
            for step in range(8):
                next(sg)
                unit_vt(1, step, pool_tag="ot")()
            av_prev = av_stream(0, 0, es)

            for (b, c) in ((0, 1), (0, 2)):
                es = []
                sg = scores_stream(b, c, es)
                for _ in range(8):
                    next(sg)
                    next(av_prev, None)
                for _ in av_prev:
                    pass
                av_prev = av_stream(b, c, es)

            # mid block: AV(P2) interleaved with the dense b1 QKV block
            units_mid = [unit_k(1, 0, 0, a), unit_k(1, 0, 1, v), unit_q(1, 0),
                         unit_k(1, 1, 0, a), unit_k(1, 1, 1, v), unit_q(1, 1),
                         unit_k(1, 2, 0, a), unit_k(1, 2, 1, v), unit_q(1, 2)]
            for i in range(8):
                next(av_prev, None)
                units_mid[i]()
            units_mid[8]()
            for _ in av_prev:
                pass

            # P3 scores with proj(b0) interleaved (ot-tag slots)
            es = []
            sg = scores_stream(1, 0, es)
            for step in range(8):
                next(sg)
                if step < 3:
                    unit_proj(0, step, pool_tag="ot")()
            av_prev = av_stream(1, 0, es)

            for (b, c) in ((1, 1), (1, 2)):
                es = []
                sg = scores_stream(b, c, es)
                for _ in range(8):
                    next(sg)
                    next(av_prev, None)
                for _ in av_prev:
                    pass
                av_prev = av_stream(b, c, es)
            for _ in av_prev:
                pass
            for m in range(3):
                unit_proj(1, m)()

    nc.compile()
    return nc


def _build_inputs(x, w_qkv, b_qkv, w_proj, b_proj, attn_idx):
    bf = ml_dtypes.bfloat16
    f8 = ml_dtypes.float8_e4m3
    x = np.asarray(x, np.float32)
    w_qkv = np.asarray(w_qkv, np.float32)
    b_qkv = np.asarray(b_qkv, np.float32)
    w_proj = np.asarray(w_proj, np.float32)
    b_proj = np.asarray(b_proj, np.float32)
    attn_idx = np.asarray(attn_idx).astype(np.int64)

    wqkvT = np.ascontiguousarray(w_qkv.T)  # [in, out]; q 0:C k C:2C v 2C:
    blocks = []
    for c in range(3):
        blocks.append(wqkvT[:, C + 128 * c:C + 128 * (c + 1)] * W8S)  # K(c)
        blocks.append(wqkvT[:, 128 * c:128 * (c + 1)] * (W8S * SCALE))  # Q(c)
    w8 = np.ascontiguousarray(np.concatenate(blocks, axis=1)).astype(f8)
    wv = np.ascontiguousarray(wqkvT[:, 2 * C:]).astype(bf)

    bq = np.ascontiguousarray(
        (b_qkv[:C] * SCALE).reshape(3, 128).T).astype(np.float32)
    bp = np.ascontiguousarray(
        (b_proj + w_proj @ b_qkv[2 * C:]).reshape(3, 128).T).astype(np.float32)
    wprojT = np.ascontiguousarray(w_proj.T).astype(bf)

    in_maps = []
    for i in range(NCORES):
        sr = slab_rows_for_core(i)
        slab = np.zeros((B, C, SLAB, WW), np.float32)
        for s, r in enumerate(sr):
            if r != PAD:
                slab[:, :, s, :] = x[:, :, r, :]
        slab = np.ascontiguousarray(slab.reshape(B, C, SCOLS))

        row2slot = np.full((4, HH), -1, np.int64)
        for t in range(4):
            for s in range(2 * t + 9, 2 * t - 1, -1):
                if sr[s] != PAD:
                    row2slot[t, sr[s]] = s
        q0 = 8 * i * WW
        aidx = attn_idx[q0:q0 + QCOLS]          # [512, 33]
        t_of_q = np.arange(QCOLS) // 128
        ar = aidx // WW
        ac = aidx % WW
        slot = row2slot[t_of_q[:, None], ar]
        assert (slot >= 0).all(), f"core {i}: target row outside band"
        lidx = slot * 64 + ac
        j = lidx // 128
        kin = lidx % 128
        qq = np.repeat(np.arange(QCOLS), aidx.shape[1])
        jf = j.ravel()
        col = (np.asarray(OFF1)[jf] + (qq - 128 * np.asarray(T0_J)[jf]))
        mask = np.zeros((128, 2, MASKW), np.float32)
        mask[kin.ravel(), 0, col] = 1.0
        mask[:, 1, :] = mask[:, 0, :]
        assert int(mask[:, 0, :].sum()) == QCOLS * aidx.shape[1], f"core {i}"

        in_maps.append({
            "xs": slab.astype(bf),
            "x8": slab.astype(f8),
            "w8": w8,
            "wv": wv,
            "bq": bq,
            "wprojT": wprojT,
            "bp": bp,
            "mask": np.ascontiguousarray(mask).astype(bf),
        })
    return in_maps


def kernel(x, w_qkv, b_qkv, w_proj, b_proj, attn_idx):
    global LAST_EXEC_NS, LAST_TRACE
    _register_ntff_hook()
    if "graph" not in _NC_CACHE:
        _NC_CACHE["graph"] = build_graph()
    nc = _NC_CACHE["graph"]
    in_maps = _build_inputs(x, w_qkv, b_qkv, w_proj, b_proj, attn_idx)
    trace = bool(int(os.environ.get("BASSK_TRACE", "0")))
    res = run_bass_kernel_spmd(nc, in_maps, core_ids=list(range(NCORES)),
                               trace=trace)
    LAST_EXEC_NS = res.exec_time_ns
    if res.instructions_and_trace is not None:
        LAST_TRACE = res.instructions_and_trace[1]
    out = np.empty((B, C, HH, WW), np.float32)
    for i in range(NCORES):
        o = res.results[i]["out"].reshape(B, C, ROWS, WW)
        out[:, :, 8 * i:8 * i + ROWS, :] = o
    return out


# revision 21
# speedup vs baseline: 1.1909x; 1.1909x over previous
"""Trainium2 Bass kernel for nn_ALAttention (sparse local attention).

Sharding: 64 image rows split across 8 cores (8 query rows each). All 33
attention targets of a query in row r lie within rows r-4..r+4, so each core
works on a host-sliced 16-row halo slab of x (virtually centered, zero-padded
at borders -> identical SPMD graph; padded keys are masked out). Per core:
QKV GEMM (bf16, fused bias, q-scale folded into host-prescaled bias), masked
dense local attention in S^T=[keys,q] layout (host-built mask from attn_idx,
exp -> mask-mult -> V_aug matmul whose ones-column yields the softmax
denominator), normalize via fast-reciprocal + partition-broadcast, proj GEMM.
Key chunks 0 and 7 of the 1024-key slab are only needed by the first/last
query row-pair (host-asserted), so their score/exp/AV work runs at 1/4 width.
No inter-core communication.
"""
import os
import sys
import types

sys.path.insert(0, "/opt/trn_rl_repo")

import numpy as np
import ml_dtypes

from concourse import bacc, tile, mybir
from concourse import bass_utils
from concourse import masks as bass_masks
from concourse.bass_utils import run_bass_kernel_spmd

F32 = mybir.dt.float32
BF16 = mybir.dt.bfloat16
AF = mybir.ActivationFunctionType
ALU = mybir.AluOpType

B = 2
C = 384
HH = WW = 64
HEADS = 6
NCORES = 8
ROWS = 8
SLAB = 16
SCOLS = SLAB * WW      # 1024 slab key positions
QCOLS = ROWS * WW      # 512 queries per core
NKC = SCOLS // 128     # 8 key chunks
SCALE = float(64) ** -0.5
# full-width key chunks (1..6) run for all 512 queries; chunks 0 and 7 are
# banded (only the first/last query row-pair needs them, host-asserted)
FULL_CHUNKS = (1, 2, 3, 4, 5, 6)
MASKW = 6 * 512 + 256

LAST_EXEC_NS = None
LAST_TRACE = None
_NC_CACHE = {}


def _register_ntff_hook():
    if "antenv.axon_hooks" in sys.modules:
        return
    try:
        from trn_agent_boot.trn_boot import _ntff_profile_via_ctypes
        hook = _ntff_profile_via_ctypes("/opt/axon/libaxon_pjrt.so")
    except Exception:
        hook = None
    mod = types.ModuleType("antenv.axon_hooks")
    mod.get_axon_ntff_profile_hook = lambda: hook
    mod.set_axon_ntff_profile_hook = lambda h: None
    sys.modules["antenv.axon_hooks"] = mod
    bass_utils.upload_artifacts = lambda tmpdir: "local://skipped"


def build_graph():
    nc = bacc.Bacc("TRN2", target_bir_lowering=False, debug=False,
                   num_devices=NCORES)

    xs_e = nc.dram_tensor("xs", [B, C, SCOLS], BF16, kind="ExternalInput").ap()
    wqkvT_e = nc.dram_tensor("wqkvT", [C, 3 * C], BF16, kind="ExternalInput").ap()
    bqkv_e = nc.dram_tensor("bqkv", [128, 9], F32, kind="ExternalInput").ap()
    wprojT_e = nc.dram_tensor("wprojT", [C, C], BF16, kind="ExternalInput").ap()
    bproj_e = nc.dram_tensor("bproj", [128, 3], F32, kind="ExternalInput").ap()
    mask_e = nc.dram_tensor("mask", [128, MASKW], BF16,
                            kind="ExternalInput").ap()
    out_e = nc.dram_tensor("out", [B, C, QCOLS], F32, kind="ExternalOutput").ap()

    with tile.TileContext(nc) as tc:
        with (
            tc.tile_pool(name="const", bufs=1) as cpool,
            tc.tile_pool(name="xin", bufs=2) as xpool,
            tc.tile_pool(name="qkv", bufs=2) as qkvpool,
            tc.tile_pool(name="vt", bufs=2) as vtpool,
            tc.tile_pool(name="esb", bufs=3) as epool,
            tc.tile_pool(name="osb", bufs=2) as opool,
            tc.tile_pool(name="sc", bufs=3) as scpool,
            tc.tile_pool(name="psA", bufs=2, space="PSUM") as psA,
            tc.tile_pool(name="psB", bufs=4, space="PSUM") as psB,
        ):
            # interleave x/w chunk DMAs so the first matmul is gated only by
            # its own chunks; masks go via the gpsimd SWDGE queue in parallel
            x_sb0 = xpool.tile([128, 3, SCOLS], BF16, tag="x", name="x_sb0")
            w0_sb = cpool.tile([128, 3, 128], BF16, tag="wqkv0")
            w_sb = cpool.tile([128, 3, 3 * C], BF16, tag="wqkv")
            qs = [nc.sync, nc.scalar, nc.gpsimd]
            for k in range(3):
                qs[k].dma_start(x_sb0[:, k, :], xs_e[0, 128 * k:128 * (k + 1), :])
                qs[(k + 1) % 3].dma_start(w0_sb[:, k, :],
                                          wqkvT_e[128 * k:128 * (k + 1), 0:128])
            for k in range(3):
                qs[k].dma_start(w_sb[:, k, :],
                                wqkvT_e[128 * k:128 * (k + 1), :])
            bq_sb = cpool.tile([128, 9], F32, tag="bqkv")
            nc.sync.dma_start(bq_sb[:], bqkv_e[:])
            bp_sb = cpool.tile([128, 3], F32, tag="bproj")
            nc.sync.dma_start(bp_sb[:], bproj_e[:])
            ident = cpool.tile([128, 128], BF16, tag="ident")
            bass_masks.make_identity(nc, ident[:])
            # pre-warm the scalar engine's EXP table
            warm_sb = cpool.tile([1, 1], F32, tag="warm")
            nc.scalar.activation(warm_sb[:], ident[0:1, 0:1], AF.Exp)
            mask_sb = cpool.tile([128, MASKW], BF16, tag="mask")
            nc.gpsimd.dma_start(mask_sb[:], mask_e[:])
            wp_sb = cpool.tile([128, 3, C], BF16, tag="wproj")
            for k in range(3):
                nc.scalar.dma_start(wp_sb[:, k, :],
                                    wprojT_e[128 * k:128 * (k + 1), :])

            # ---- QKV GEMMs for BOTH batches first: one long dense PE run
            # that warms the HAM clock gate and stays ahead of attention ----
            qkv_mb = []
            for b in range(B):
                if b == 0:
                    x_sb = x_sb0
                else:
                    x_sb = xpool.tile([128, 3, SCOLS], BF16, tag="x",
                                      name="x_sb1")
                    for k in range(3):
                        nc.gpsimd.dma_start(x_sb[:, k, :],
                                            xs_e[b, 128 * k:128 * (k + 1), :])

                qkv_m = [qkvpool.tile([128, SCOLS], BF16, tag=f"qkv{m}",
                                      name=f"qkv{m}_{b}") for m in range(9)]
                qkv_mb.append(qkv_m)
                for m in (0, 3, 6, 1, 4, 7, 2, 5, 8):
                    is_q = m < 3
                    ps = psA.tile([128, 1024], F32, tag="mm")
                    if is_q:
                        for k in range(3):
                            nc.tensor.matmul(
                                ps[:, 0:512],
                                w0_sb[:, k, :] if m == 0
                                else w_sb[:, k, 128 * m:128 * (m + 1)],
                                x_sb[:, k, 256:768],
                                start=(k == 0), stop=(k == 2))
                        nc.scalar.activation(
                            qkv_m[m][:, 256:768], ps[:, 0:512],
                            AF.Identity, bias=bq_sb[:, m:m + 1], scale=SCALE)
                    else:
                        for n in range(2):
                            for k in range(3):
                                nc.tensor.matmul(
                                    ps[:, 512 * n:512 * (n + 1)],
                                    w_sb[:, k, 128 * m:128 * (m + 1)],
                                    x_sb[:, k, 512 * n:512 * (n + 1)],
                                    start=(k == 0), stop=(k == 2))
                        if m % 2 == 0:
                            nc.scalar.activation(
                                qkv_m[m][:], ps[:],
                                AF.Identity, bias=bq_sb[:, m:m + 1], scale=1.0)
                        else:
                            nc.vector.tensor_scalar(
                                qkv_m[m][:], ps[:], bq_sb[:, m:m + 1], None,
                                ALU.add)

            ocats = [[opool.tile([128, QCOLS], BF16, tag=f"ocat{c}",
                                 name=f"ocat{c}_{b}") for c in range(3)]
                     for b in range(B)]
            v_sbs = {}

            def emit_vtrans_pair(b, c):
                v_t = qkv_mb[b][6 + c]
                v_ps = [psB.tile([128, NKC, 64], BF16, tag="acc",
                                 name=f"v_ps{c}{eo}_{b}")
                        for eo in range(2)]
                for j in range(NKC):
                    for eo in range(2):
                        mo = 64 * eo
                        nc.tensor.transpose(
                            v_ps[eo][:, j, :],
                            v_t[mo:mo + 64, 128 * j:128 * (j + 1)],
                            ident[mo:mo + 64, mo:mo + 64])
                for eo in range(2):
                    v_sb = vtpool.tile([128, NKC, 128], BF16,
                                       tag=f"vt{2 * c + eo}",
                                       name=f"v_sb{2 * c + eo}_{b}")
                    nc.vector.tensor_copy(v_sb[:, :, 0:64], v_ps[eo][:])
                    nc.gpsimd.memset(v_sb[:, :, 64:65], 1.0)
                    v_sbs[(b, 2 * c + eo)] = v_sb

            def emit_pair(b, c, filler):
                qkv_m = qkv_mb[b]
                ocat = ocats[b]
                k_t = qkv_m[3 + c]
                q_e = qkv_m[c][0:64, 256:768]
                q_o = qkv_m[c][64:128, 256:768]
                v_e, v_o = v_sbs[(b, 2 * c)], v_sbs[(b, 2 * c + 1)]

                ot_e = psB.tile([128, QCOLS], F32, tag="acc",
                                name=f"ot_e{c}_{b}")
                ot_o = psB.tile([128, QCOLS], F32, tag="acc",
                                name=f"ot_o{c}_{b}")
                for gi, j in enumerate(FULL_CHUNKS):
                    st = psA.tile([128, 2, 512], F32, tag="mm",
                                  name=f"st{c}_{gi}_{b}")
                    if filler:
                        # density filler: overwritten by the start=True
                        # matmul below; keeps the PE activity monitor warm
                        nc.tensor.matmul(
                            st[:, 0, 0:512], k_t[0:64, 0:128],
                            q_e, start=True, stop=True)
                    nc.tensor.matmul(
                        st[:, 0, :],
                        k_t[0:64, 128 * j:128 * (j + 1)],
                        q_e, start=True, stop=True)
                    nc.tensor.matmul(
                        st[:, 1, :],
                        k_t[64:128, 128 * j:128 * (j + 1)],
                        q_o, start=True, stop=True)
                    e_sb = epool.tile([128, 2, 512], BF16, tag="e",
                                      name=f"e{c}_{gi}_{b}")
                    nc.scalar.activation(e_sb[:], st[:], AF.Exp)
                    nc.vector.tensor_tensor(
                        e_sb[:], e_sb[:],
                        mask_sb[:, 512 * gi:512 * (gi + 1)][:, None, :]
                        .to_broadcast([128, 2, 512]),
                        ALU.mult)
                    nc.tensor.matmul(
                        ot_e[:], v_e[:, j, :], e_sb[:, 0, :],
                        start=(gi == 0), stop=False,
                        skip_group_check=True)
                    nc.tensor.matmul(
                        ot_o[:], v_o[:, j, :], e_sb[:, 1, :],
                        start=(gi == 0), stop=False,
                        skip_group_check=True)

                # banded tail: chunk 0 -> queries 0:128, chunk 7 ->
                # queries 384:512 (per head)
                for eo, (q_h, v_h, ot_h) in enumerate(
                        ((q_e, v_e, ot_e), (q_o, v_o, ot_o))):
                    mo = 64 * eo
                    st = psA.tile([128, 2, 512], F32, tag="mm",
                                  name=f"stt{c}{eo}_{b}")
                    nc.tensor.matmul(st[:, 0, 0:128],
                                     k_t[mo:mo + 64, 0:128],
                                     q_h[:, 0:128], start=True, stop=True)
                    nc.tensor.matmul(st[:, 0, 128:256],
                                     k_t[mo:mo + 64, 896:1024],
                                     q_h[:, 384:512], start=True,
                                     stop=True)
                    e_sb = epool.tile([128, 2, 512], BF16, tag="e",
                                      name=f"et{c}{eo}_{b}")
                    nc.scalar.activation(e_sb[:, 0, 0:256],
                                         st[:, 0, 0:256], AF.Exp)
                    nc.vector.tensor_tensor(
                        e_sb[:, 0, 0:256], e_sb[:, 0, 0:256],
                        mask_sb[:, 3072:3328], ALU.mult)
                    nc.tensor.matmul(ot_h[:, 0:128], v_h[:, 0, :],
                                     e_sb[:, 0, 0:128], start=False,
                                     stop=False, skip_group_check=True)
                    nc.tensor.matmul(ot_h[:, 384:512], v_h[:, 7, :],
                                     e_sb[:, 0, 128:256], start=False,
                                     stop=True, skip_group_check=True)

                for eo, ot in ((0, ot_e), (1, ot_o)):
                    mo = 64 * eo
                    srow = scpool.tile([1, QCOLS], F32, tag="srow")
                    nc.vector.tensor_copy(srow[:], ot[64:65, :])
                    rrow = scpool.tile([1, QCOLS], F32, tag="rrow")
                    nc.vector.reciprocal_approx_fast(rrow[:], srow[:])
                    rb = scpool.tile([64, QCOLS], F32, tag="rb")
                    nc.gpsimd.partition_broadcast(rb[:], rrow[:])
                    nc.vector.tensor_tensor(
                        ocat[c][mo:mo + 64, :], ot[0:64, :], rb[:],
                        ALU.mult)

            def emit_proj(b):
                ocat = ocats[b]
                pps = [psB.tile([128, QCOLS], F32, tag="acc",
                                name=f"pp{m}_{b}") for m in range(3)]
                for k in range(3):
                    for m in range(3):
                        nc.tensor.matmul(
                            pps[m][:], wp_sb[:, k, 128 * m:128 * (m + 1)],
                            ocat[k][:], start=(k == 0), stop=(k == 2),
                            skip_group_check=True)
                outq = [nc.sync, nc.scalar, nc.gpsimd]
                for m in range(3):
                    o_sb = scpool.tile([128, QCOLS], F32, tag="out")
                    if m == 1:
                        nc.vector.tensor_scalar(
                            o_sb[:], pps[m][:], bp_sb[:, m:m + 1], None,
                            ALU.add)
                    else:
                        nc.scalar.activation(
                            o_sb[:], pps[m][:], AF.Identity,
                            bias=bp_sb[:, m:m + 1], scale=1.0)
                    outq[m].dma_start(out_e[b, 128 * m:128 * (m + 1), :],
                                      o_sb[:])

            # b0 vtrans as warm anchor; b1 vtrans pairs interleaved into
            # b0's attention as REAL density filler (replaces the artificial
            # fillers there); b1 pairs keep the artificial filler.
            for c in range(3):
                emit_vtrans_pair(0, c)
            for c in range(3):
                emit_pair(0, c, filler=False)
                emit_vtrans_pair(1, c)
            emit_proj(0)
            for c in range(3):
                emit_pair(1, c, filler=True)
            emit_proj(1)

    nc.compile()
    return nc


def _build_inputs(x, w_qkv, b_qkv, w_proj, b_proj, attn_idx):
    bf = ml_dtypes.bfloat16
    x = np.asarray(x, np.float32)
    attn_idx = np.asarray(attn_idx)

    xp = np.zeros((B, C, HH + 8, WW), np.float32)
    xp[:, :, 4:4 + HH, :] = x
    xp = xp.astype(bf)

    wqkvT = np.ascontiguousarray(np.asarray(w_qkv, np.float32).T).astype(bf)
    wprojT = np.ascontiguousarray(np.asarray(w_proj, np.float32).T).astype(bf)

    b_adj = np.asarray(b_qkv, np.float32).copy()
    b_adj[:C] *= SCALE
    bqkv = np.ascontiguousarray(b_adj.reshape(9, 128).T)
    bproj = np.ascontiguousarray(
        np.asarray(b_proj, np.float32).reshape(3, 128).T)

    in_maps = []
    for i in range(NCORES):
        slab = np.ascontiguousarray(
            xp[:, :, 8 * i:8 * i + SLAB, :]).reshape(B, C, SCOLS)
        q0 = 8 * i * WW
        gq = np.arange(q0, q0 + QCOLS)
        aidx = attn_idx[gq].astype(np.int64)
        local = aidx - (8 * i - 4) * WW
        assert local.min() >= 0 and local.max() < SCOLS, \
            f"core {i}: attn target outside slab"
        m = np.zeros((NKC, 128, QCOLS), np.float32)
        qq = np.repeat(np.arange(QCOLS), aidx.shape[1])
        ll = local.ravel()
        m[ll // 128, ll % 128, qq] = 1.0
        # banded-tail coverage: chunk 0 only serves queries 0:128,
        # chunk 7 only queries 384:512
        assert m[0, :, 128:].sum() == 0, f"core {i}: chunk0 band violated"
        assert m[7, :, :384].sum() == 0, f"core {i}: chunk7 band violated"
        # per full chunk (shared by the head pair via a step-0 broadcast
        # AP on-device); tail packed as [m0 | m7] over the banded ranges
        packed = np.zeros((128, MASKW), np.float32)
        for g, j in enumerate(FULL_CHUNKS):
            packed[:, 512 * g:512 * (g + 1)] = m[j]
        packed[:, 3072:3200] = m[0][:, 0:128]
        packed[:, 3200:3328] = m[7][:, 384:512]
        in_maps.append({
            "xs": slab,
            "wqkvT": wqkvT,
            "bqkv": bqkv,
            "wprojT": wprojT,
            "bproj": bproj,
            "mask": np.ascontiguousarray(packed).astype(bf),
        })
    return in_maps


def kernel(x, w_qkv, b_qkv, w_proj, b_proj, attn_idx):
    global LAST_EXEC_NS, LAST_TRACE
    _register_ntff_hook()
    if "graph" not in _NC_CACHE:
        _NC_CACHE["graph"] = build_graph()
    nc = _NC_CACHE["graph"]
    in_maps = _build_inputs(x, w_qkv, b_qkv, w_proj, b_proj, attn_idx)
    trace = bool(int(os.environ.get("BASSK_TRACE", "0")))
    res = run_bass_kernel_spmd(nc, in_maps, core_ids=list(range(NCORES)),
                               trace=trace)
    LAST_EXEC_NS = res.exec_time_ns
    if res.instructions_and_trace is not None:
        LAST_TRACE = res.instructions_and_trace[1]
    out = np.empty((B, C, HH, WW), np.float32)
    for i in range(NCORES):
        o = res.results[i]["out"].reshape(B, C, ROWS, WW)
        out[:, :, 8 * i:8 * i + ROWS, :] = o
    return out
